# revision 19
# baseline (speedup 1.0000x reference)
"""Trainium2 Bass kernel for nn_Actor (GNN message passing actor net).

Strategy: pure data parallelism, B=4096 sharded over 8 NeuronCores
(512 samples/core). All device compute in bf16 (inputs pre-converted on
host, halving HBM traffic). Activations live feature-major ("x^T":
features on partitions, tokens on the free dim) in an even/odd-pair
dual-stream layout: partitions 0-63 hold even samples' features,
64-127 odd samples'. Dense layers use block-diagonal duplicated weights
so one [128,128]x[128,512] matmul serves both halves at full PE width.
The per-sample adjacency matmuls run as 64x64 quadrant matmuls
(tile_position) with adjacencies pre-transposed on the host; x2 is
flipped to node-major on-chip with PE transpose-mode matmuls.
"""

import sys
import types

import numpy as np
import ml_dtypes

BF16 = ml_dtypes.bfloat16

B, N, S, H = 4096, 64, 64, 64
NCORES = 8
BL = B // NCORES          # 512 samples per core
TS = BL * N // 2          # 16384 token-columns per half
NCHUNK = 32               # chunks per core; 16 samples (1024 tokens) each
GRP = 4                   # chunks per DMA superchunk / action group

_NC_CACHE = {}


def _setup_env():
    """Shim the missing antenv.axon_hooks module so trace=True works."""
    if "antenv.axon_hooks" in sys.modules:
        return
    try:
        from trn_agent_boot.trn_boot import _ntff_profile_via_ctypes
        hook = _ntff_profile_via_ctypes("/opt/axon/libaxon_pjrt.so")
    except Exception:
        hook = None
    mod = types.ModuleType("antenv.axon_hooks")
    mod.get_axon_ntff_profile_hook = lambda: hook
    mod.set_axon_ntff_profile_hook = lambda h: None
    sys.modules["antenv.axon_hooks"] = mod


def _patch_tile():
    """This walrus build rejects instructions with >1 sem wait; split the
    Tile kernel-tail drain's waits across single-wait SP nops."""
    import concourse.mybir as mybir
    import concourse.tile as tile_mod
    from concourse.vector_clock import ScopedClock

    if getattr(tile_mod.TileContext, "_drain_patched", False):
        return

    def _drain_and_barrier(self, tick_clock, wait_clock):
        drain_inst = self.nc.sync.drain()
        wait_clock.add_sem_waits(
            drain_inst.ins, ScopedClock({None: tick_clock.global_clock})
        )
        si = drain_inst.ins.sync_info
        if si is not None and len(si.on_wait) > 1:
            waits = list(si.on_wait)
            drain_inst.ins.sync_info = mybir.SyncInfo(
                on_wait=[waits[0]], on_update=list(si.on_update)
            )
            for w in waits[1:]:
                nop = self.nc.sync.nop()
                nop.ins.sync_info = mybir.SyncInfo(on_wait=[w], on_update=[])
        self.nc.all_engine_barrier()
        popped = self.nc._tile_sem_poison_stack.pop()
        assert popped is self._sem_poison
        self.nc.clear_and_free_semaphores(list(self.sems.allocated().values()))
        self.nc.all_engine_barrier()

    tile_mod.TileContext._drain_and_barrier = _drain_and_barrier
    tile_mod.TileContext._drain_patched = True


def _split_multiwaits(nc):
    """walrus on this stack accepts only one sem-wait per instruction; hoist
    extra waits onto same-engine nops inserted immediately before."""
    import concourse.mybir as mybir

    k = 0
    for bb in nc.m.functions[0].blocks:
        new = []
        for inst in bb.instructions:
            si = inst.sync_info
            if si is not None and len(si.on_wait) > 1:
                waits = list(si.on_wait)
                for w in waits[:-1]:
                    nop = mybir.InstNoOp(
                        name=f"wsplit-{k}",
                        engine=inst.engine,
                        bass_nofuse=True,
                        sync_info=mybir.SyncInfo(on_wait=[w], on_update=[]),
                    )
                    k += 1
                    nc.register_instruction(nop, overwrite=True)
                    new.append(nop)
                inst.sync_info = mybir.SyncInfo(
                    on_wait=[waits[-1]], on_update=list(si.on_update)
                )
            new.append(inst)
        bb.instructions[:] = new


# weight column offsets inside the packed `wts` [128, 1794] bf16 tensor
W1, W2, WC, WO = 0, 128, 256, 384
W3A, W3B, W3C = 512, 640, 768
WIR, WIZ, WIN = 896, 1024, 1152
WHR, WHZ, WHN = 1280, 1408, 1536
IDN, WOUT = 1664, 1792
WCOLS = 1794
# bias column offsets inside `bias` [128, 10] f32
B1, B2, BC, BO, B3, BR, BZ, BHHN, BIHN, BOUT = range(10)


def build_nc(nchunk=NCHUNK, stages="full"):
    import concourse.bass as bass
    import concourse.mybir as mybir
    from concourse.tile import TileContext

    _patch_tile()
    TS_ = 512 * nchunk
    f32 = mybir.dt.float32
    bf16 = mybir.dt.bfloat16
    add = mybir.AluOpType.add
    mult = mybir.AluOpType.mult
    sub = mybir.AluOpType.subtract
    amax = mybir.AluOpType.max
    Sig = mybir.ActivationFunctionType.Sigmoid
    Relu = mybir.ActivationFunctionType.Relu
    Tanh = mybir.ActivationFunctionType.Tanh

    nc = bass.Bass("TRN2")
    xT = nc.declare_dram_parameter("xT", [128, TS_], bf16, isOutput=False)
    hT = nc.declare_dram_parameter("hT", [128, TS_], bf16, isOutput=False)
    adjT = nc.declare_dram_parameter("adjT", [128, 2 * TS_], bf16, isOutput=False)
    wts = nc.declare_dram_parameter("wts", [128, WCOLS], bf16, isOutput=False)
    bias = nc.declare_dram_parameter("bias", [128, 10], f32, isOutput=False)
    houtT = nc.declare_dram_parameter("houtT", [128, TS_], bf16, isOutput=True)
    aout = nc.declare_dram_parameter("aout", [8 * (nchunk // GRP), 512], bf16, isOutput=True)

    with TileContext(nc) as tc:
        with (
            tc.tile_pool(name="const", bufs=1) as cpool,
            tc.tile_pool(name="inp", bufs=2) as ipool,
            tc.tile_pool(name="work", bufs=6) as wpool,
            tc.tile_pool(name="hout", bufs=2) as hpool,
            tc.tile_pool(name="psum", bufs=7, space="PSUM") as ppool,
            tc.tile_pool(name="psumA", bufs=1, space="PSUM") as apool,
        ):
            wsb = cpool.tile([128, WCOLS], bf16)
            nc.sync.dma_start(out=wsb[:], in_=wts[:])
            bsb = cpool.tile([128, 10], f32)
            nc.sync.dma_start(out=bsb[:], in_=bias[:])

            def bias_ap(col):
                return bsb[:, col : col + 1]

            CW = 512 * GRP  # token-cols per superchunk

            for g in range(nchunk // GRP):
                xs = ipool.tile([128, CW], bf16, tag="xs")
                nc.sync.dma_start(out=xs[:], in_=xT[:, g * CW : (g + 1) * CW])
                hs = ipool.tile([128, CW], bf16, tag="hs")
                nc.sync.dma_start(out=hs[:], in_=hT[:, g * CW : (g + 1) * CW])
                ads = ipool.tile([128, 2 * CW], bf16, tag="ads")
                nc.sync.dma_start(
                    out=ads[:], in_=adjT[:, g * 2 * CW : (g + 1) * 2 * CW]
                )
                hacc = hpool.tile([128, CW], bf16, tag="hacc")
                pact = apool.tile([128, 512], f32, tag="pact")
                nc.vector.memset(pact[:], 0.0)

                _psc = [0]

                def ps():
                    _psc[0] += 1
                    return ppool.tile([128, 512], f32, tag="ps", name=f"ps_{g}_{_psc[0]}")

                # ---- Phase 1: fc1/fc2 (weight-stationary x4), transpose, bmm ----
                p1s = [ps() for _ in range(GRP)]
                for k in range(GRP):
                    nc.tensor.matmul(p1s[k][:], wsb[:, W1 : W1 + 128],
                                     xs[:, 512 * k : 512 * (k + 1)], start=True, stop=True)
                x1s = []
                for k in range(GRP):
                    x1 = wpool.tile([128, 512], bf16, tag="x1")
                    nc.scalar.activation(x1[:], p1s[k][:], Relu, bias=bias_ap(B1))
                    x1s.append(x1)
                p2s = [ps() for _ in range(GRP)]
                for k in range(GRP):
                    nc.tensor.matmul(p2s[k][:], wsb[:, W2 : W2 + 128], x1s[k][:], start=True, stop=True)
                x2s = []
                for k in range(GRP):
                    x2 = wpool.tile([128, 512], bf16, tag="x2")
                    nc.scalar.activation(x2[:], p2s[k][:], Relu, bias=bias_ap(B2))
                    x2s.append(x2)
                if stages == "fc2":
                    for k in range(GRP):
                        nc.gpsimd.tensor_copy(out=hacc[:, 512 * k : 512 * (k + 1)], in_=x2s[k][:])
                    nc.sync.dma_start(out=houtT[:, g * CW : (g + 1) * CW], in_=hacc[:])
                    continue

                Msbs = []
                for k in range(GRP):
                    x2 = x2s[k]
                    acol = ads[:, 1024 * k : 1024 * (k + 1)]
                    # x2 -> node-major via PE transpose-mode; row-group-64
                    # transposes need a different PSUM bank than row-group-0
                    # (concurrent same-bank writes from different row groups
                    # hard-fault the PSUM).
                    pTe = ppool.tile([128, 256], bf16, tag="ps", name=f"pTe_{g}_{k}")
                    pTo = ppool.tile([128, 256], bf16, tag="ps", name=f"pTo_{g}_{k}")
                    for s in range(4):
                        nc.tensor.transpose(
                            pTe[:, 64 * s : 64 * s + 64],
                            x2[0:64, 128 * s : 128 * (s + 1)],
                            wsb[0:64, IDN : IDN + 64],
                            tile_position=(0, 0),
                        )
                        nc.tensor.transpose(
                            pTo[:, 64 * s : 64 * s + 64],
                            x2[64:128, 128 * s : 128 * (s + 1)],
                            wsb[64:128, IDN + 64 : IDN + 128],
                            tile_position=(64, 0),
                        )
                    x2ne = wpool.tile([128, 256], bf16, tag="x2ne")
                    nc.vector.tensor_copy(out=x2ne[:], in_=pTe[:])
                    x2no = wpool.tile([128, 256], bf16, tag="x2no")
                    nc.vector.tensor_copy(out=x2no[:], in_=pTo[:])

                    # adjacency matmuls: M = [A_c @ X2 | A_o @ X2]^T per sample;
                    # separate psum banks per lhsT row group.
                    pM0 = ppool.tile([128, 512], f32, tag="ps", name=f"pM0_{g}_{k}")
                    pM1 = ppool.tile([128, 512], f32, tag="ps", name=f"pM1_{g}_{k}")
                    for m in range(16):
                        s, r = m // 4, m % 4
                        rb = 64 * (r // 2)
                        cb = 64 * (r % 2)
                        x2n = x2ne if r % 2 == 0 else x2no
                        pM = pM0 if r < 2 else pM1
                        lhs = x2n[rb : rb + 64, 64 * s : 64 * s + 64]
                        rhs = acol[rb : rb + 64, 256 * s + 128 * (r % 2) : 256 * s + 128 * (r % 2) + 128]
                        nc.tensor.matmul(
                            pM[cb : cb + 64, 128 * s : 128 * s + 128],
                            lhs, rhs, start=True, stop=True,
                            tile_position=(rb, cb),
                        )
                    Msb = wpool.tile([128, 1024], bf16, tag="Msb")
                    M2 = Msb.rearrange("p (a c) -> p a c", a=4, c=256)
                    nc.vector.tensor_copy(out=M2[:, :, 0:128], in_=pM0[:])
                    nc.vector.tensor_copy(out=M2[:, :, 128:256], in_=pM1[:])
                    Msbs.append(Msb)
                if stages in ("xpose", "bmm"):
                    for k in range(GRP):
                        nc.gpsimd.tensor_copy(out=hacc[:, 512 * k : 512 * (k + 1)], in_=Msbs[k][:, 0:512])
                    nc.sync.dma_start(out=houtT[:, g * CW : (g + 1) * CW], in_=hacc[:])
                    continue

                # ---- Phase 2: comp/coop linears + fc3 (weight-stationary x4) ----
                M4s = [Msbs[k].rearrange("p (q two h) -> p q two h", two=2, h=64) for k in range(GRP)]
                pcs = [ps() for _ in range(GRP)]
                for k in range(GRP):
                    nc.tensor.matmul(pcs[k][:], wsb[:, WC : WC + 128], M4s[k][:, :, 0, :], start=True, stop=True)
                xcs = []
                for k in range(GRP):
                    xc = wpool.tile([128, 512], bf16, tag="xc")
                    nc.scalar.activation(xc[:], pcs[k][:], Relu, bias=bias_ap(BC))
                    xcs.append(xc)
                pos = [ps() for _ in range(GRP)]
                for k in range(GRP):
                    nc.tensor.matmul(pos[k][:], wsb[:, WO : WO + 128], M4s[k][:, :, 1, :], start=True, stop=True)
                xos = []
                for k in range(GRP):
                    xo = wpool.tile([128, 512], bf16, tag="xo")
                    nc.vector.tensor_scalar(xo[:], pos[k][:], bias_ap(BO), 0.0, add, amax)
                    xos.append(xo)
                p3s = [ps() for _ in range(GRP)]
                for k in range(GRP):
                    nc.tensor.matmul(p3s[k][:], wsb[:, W3A : W3A + 128], x2s[k][:], start=True, stop=False)
                for k in range(GRP):
                    nc.tensor.matmul(p3s[k][:], wsb[:, W3B : W3B + 128], xcs[k][:], start=False, stop=False)
                for k in range(GRP):
                    nc.tensor.matmul(p3s[k][:], wsb[:, W3C : W3C + 128], xos[k][:], start=False, stop=True)
                x3s = []
                for k in range(GRP):
                    x3 = wpool.tile([128, 512], bf16, tag="x3")
                    nc.vector.tensor_scalar(x3[:], p3s[k][:], bias_ap(B3), 0.0, add, amax)
                    x3s.append(x3)
                if stages == "fc3":
                    for k in range(GRP):
                        nc.gpsimd.tensor_copy(out=hacc[:, 512 * k : 512 * (k + 1)], in_=x3s[k][:])
                    nc.sync.dma_start(out=houtT[:, g * CW : (g + 1) * CW], in_=hacc[:])
                    continue

                # ---- Phase 3: GRU + actions (weight-stationary x4) ----
                hcols = [hs[:, 512 * k : 512 * (k + 1)] for k in range(GRP)]
                prs = [ps() for _ in range(GRP)]
                for k in range(GRP):
                    nc.tensor.matmul(prs[k][:], wsb[:, WIR : WIR + 128], x3s[k][:], start=True, stop=False)
                for k in range(GRP):
                    nc.tensor.matmul(prs[k][:], wsb[:, WHR : WHR + 128], hcols[k], start=False, stop=True)
                rs = []
                for k in range(GRP):
                    r = wpool.tile([128, 512], bf16, tag="r")
                    nc.scalar.activation(r[:], prs[k][:], Sig, bias=bias_ap(BR))
                    rs.append(r)
                pzs = [ps() for _ in range(GRP)]
                for k in range(GRP):
                    nc.tensor.matmul(pzs[k][:], wsb[:, WIZ : WIZ + 128], x3s[k][:], start=True, stop=False)
                for k in range(GRP):
                    nc.tensor.matmul(pzs[k][:], wsb[:, WHZ : WHZ + 128], hcols[k], start=False, stop=True)
                zs = []
                for k in range(GRP):
                    z = wpool.tile([128, 512], bf16, tag="z")
                    nc.scalar.activation(z[:], pzs[k][:], Sig, bias=bias_ap(BZ))
                    zs.append(z)
                # n = tanh(gi_n + bihn + r*(gh_n + bhhn)), two chunks at a time
                ns = []
                for half in range(2):
                    ks = [2 * half, 2 * half + 1]
                    pghs = {k: ps() for k in ks}
                    for k in ks:
                        nc.tensor.matmul(pghs[k][:], wsb[:, WHN : WHN + 128], hcols[k], start=True, stop=True)
                    pgis = {k: ps() for k in ks}
                    for k in ks:
                        nc.tensor.matmul(pgis[k][:], wsb[:, WIN : WIN + 128], x3s[k][:], start=True, stop=True)
                    for k in ks:
                        t2 = wpool.tile([128, 512], bf16, tag="t2")
                        nc.vector.scalar_tensor_tensor(t2[:], pghs[k][:], bias_ap(BHHN), rs[k][:], add, mult)
                        t3 = wpool.tile([128, 512], bf16, tag="t3")
                        nc.vector.tensor_tensor(out=t3[:], in0=t2[:], in1=pgis[k][:], op=add)
                        n = wpool.tile([128, 512], bf16, tag="n")
                        nc.scalar.activation(n[:], t3[:], Tanh, bias=bias_ap(BIHN))
                        ns.append(n)
                for k in range(GRP):
                    u = wpool.tile([128, 512], bf16, tag="u")
                    nc.vector.tensor_tensor(out=u[:], in0=hcols[k], in1=ns[k][:], op=sub)
                    v = wpool.tile([128, 512], bf16, tag="v")
                    nc.gpsimd.tensor_tensor(out=v[:], in0=zs[k][:], in1=u[:], op=mult)
                    nc.gpsimd.tensor_tensor(
                        out=hacc[:, 512 * k : 512 * (k + 1)], in0=ns[k][:], in1=v[:], op=add
                    )
                for k in range(GRP):
                    nc.tensor.matmul(
                        pact[32 * k : 32 * k + 2, :],
                        wsb[:, WOUT : WOUT + 2], x3s[k][:],
                        start=True, stop=True, tile_position=(0, 32 * k),
                    )

                # flush group: h rows and action sigmoid
                nc.sync.dma_start(out=houtT[:, g * CW : (g + 1) * CW], in_=hacc[:])
                asc = wpool.tile([128, 512], bf16, tag="asc")
                nc.scalar.activation(asc[:], pact[:], Sig, bias=bsb[:, BOUT : BOUT + 1])
                for k in range(GRP):
                    nc.sync.dma_start(
                        out=aout[8 * g + 2 * k : 8 * g + 2 * k + 2, :],
                        in_=asc[32 * k : 32 * k + 2, :],
                    )

    _split_multiwaits(nc)
    return nc


def _prep_core_inputs(state, adj_comp, adj_coop, hidden, weights):
    """Build one core's in_map from its f32 shard (all layout work on host)."""
    def eo_T(x):  # [BL,64,64] -> [128, TS] even/odd feature-major
        a = x[0::2].transpose(2, 0, 1).reshape(64, TS)
        b = x[1::2].transpose(2, 0, 1).reshape(64, TS)
        return np.concatenate([a, b], axis=0).astype(BF16)

    acT = adj_comp.transpose(0, 2, 1)
    aoT = adj_coop.transpose(0, 2, 1)
    blk = np.concatenate([acT, aoT], axis=2)  # [BL, 64, 128]
    # sample m = 16c + 4s + 2*rowhalf + q  ->  adjT[64*rh:, 1024c+256s+128q]
    blk = blk.reshape(NCHUNK, 4, 2, 2, 64, 128)
    # adjT.reshape(2, 64, NCHUNK, 4, 2, 128)[rh, j, c, s, q, col]
    adjt = blk.transpose(2, 4, 0, 1, 3, 5).reshape(128, 2 * TS)
    return {
        "xT": eo_T(state),
        "hT": eo_T(hidden),
        "adjT": adjt.astype(BF16),
        "wts": weights["wts"],
        "bias": weights["bias"],
    }


def _prep_weights(fc1_w, fc1_b, fc2_w, fc2_b, comp_w, comp_b, coop_w, coop_b,
                  fc3_w, fc3_b, out_w, out_b, gru_wih, gru_whh, gru_bih, gru_bhh):
    def bd(w):  # blockdiag of transposed layer weight [out,in] -> [128, 128]
        wt = w.T.astype(np.float32)
        z = np.zeros((128, 128), np.float32)
        z[:64, :64] = wt
        z[64:, 64:] = wt
        return z

    cols = np.zeros((128, WCOLS), np.float32)
    cols[:, W1:W1 + 128] = bd(fc1_w)
    cols[:, W2:W2 + 128] = bd(fc2_w)
    cols[:, WC:WC + 128] = bd(comp_w)
    cols[:, WO:WO + 128] = bd(coop_w)
    cols[:, W3A:W3A + 128] = bd(fc3_w[:, 0:64])
    cols[:, W3B:W3B + 128] = bd(fc3_w[:, 64:128])
    cols[:, W3C:W3C + 128] = bd(fc3_w[:, 128:192])
    cols[:, WIR:WIR + 128] = bd(gru_wih[0:64])
    cols[:, WIZ:WIZ + 128] = bd(gru_wih[64:128])
    cols[:, WIN:WIN + 128] = bd(gru_wih[128:192])
    cols[:, WHR:WHR + 128] = bd(gru_whh[0:64])
    cols[:, WHZ:WHZ + 128] = bd(gru_whh[64:128])
    cols[:, WHN:WHN + 128] = bd(gru_whh[128:192])
    cols[:, IDN:IDN + 128] = bd(np.eye(64, dtype=np.float32).T)
    cols[:64, WOUT] = out_w[0]
    cols[64:, WOUT + 1] = out_w[0]

    bias = np.zeros((128, 10), np.float32)
    for col, vec in [(B1, fc1_b), (B2, fc2_b), (BC, comp_b), (BO, coop_b),
                     (B3, fc3_b), (BR, gru_bih[0:64] + gru_bhh[0:64]),
                     (BZ, gru_bih[64:128] + gru_bhh[64:128]),
                     (BHHN, gru_bhh[128:192]), (BIHN, gru_bih[128:192])]:
        bias[:64, col] = vec
        bias[64:, col] = vec
    bias[:, BOUT] = out_b[0]
    return {"wts": cols.astype(BF16), "bias": bias}


def _decode_outputs(results):
    """results: per-core dicts with houtT [128, TS] bf16, aout [64, 512] bf16."""
    h_full = np.empty((B, N, H), np.float32)
    a_full = np.empty((B, N, 1), np.float32)
    for i, res in enumerate(results):
        ht = np.asarray(res["houtT"]).astype(np.float32)
        hs = np.empty((BL, N, H), np.float32)
        hs[0::2] = ht[:64].reshape(64, BL // 2, 64).transpose(1, 2, 0)
        hs[1::2] = ht[64:].reshape(64, BL // 2, 64).transpose(1, 2, 0)
        h_full[i * BL : (i + 1) * BL] = hs
        ao = np.asarray(res["aout"]).astype(np.float32)
        # row = 8g + 2k + parity ; col = 64q + j ; sample = 64g+16k+2q+parity
        ar = ao.reshape(8, 4, 2, 8, 64).transpose(0, 1, 3, 2, 4).reshape(BL, 64)
        a_full[i * BL : (i + 1) * BL, :, 0] = ar
    return a_full, h_full


def kernel(state, adj_comp, adj_coop, hidden,
           fc1_w, fc1_b, fc2_w, fc2_b, comp_w, comp_b, coop_w, coop_b,
           fc3_w, fc3_b, out_w, out_b, gru_wih, gru_whh, gru_bih, gru_bhh,
           trace=False):
    _setup_env()
    from concourse.bass_utils import run_bass_kernel_spmd

    if "nc" not in _NC_CACHE:
        _NC_CACHE["nc"] = build_nc()
    nc = _NC_CACHE["nc"]

    weights = _prep_weights(
        np.asarray(fc1_w), np.asarray(fc1_b), np.asarray(fc2_w), np.asarray(fc2_b),
        np.asarray(comp_w), np.asarray(comp_b), np.asarray(coop_w), np.asarray(coop_b),
        np.asarray(fc3_w), np.asarray(fc3_b), np.asarray(out_w), np.asarray(out_b),
        np.asarray(gru_wih), np.asarray(gru_whh), np.asarray(gru_bih), np.asarray(gru_bhh),
    )
    state = np.asarray(state, np.float32)
    adj_comp = np.asarray(adj_comp, np.float32)
    adj_coop = np.asarray(adj_coop, np.float32)
    hidden = np.asarray(hidden, np.float32)

    in_maps = [
        _prep_core_inputs(
            state[i * BL : (i + 1) * BL], adj_comp[i * BL : (i + 1) * BL],
            adj_coop[i * BL : (i + 1) * BL], hidden[i * BL : (i + 1) * BL], weights
        )
        for i in range(NCORES)
    ]
    res = run_bass_kernel_spmd(nc, in_maps, core_ids=list(range(NCORES)), trace=trace)
    a_full, h_full = _decode_outputs(res.results)
    kernel.last_exec_time_ns = res.exec_time_ns
    return a_full, h_full


# revision 20
# speedup vs baseline: 1.1162x; 1.1162x over previous
"""Trainium2 Bass kernel for nn_Actor (GNN message passing actor net).

Strategy: pure data parallelism, B=4096 sharded over 8 NeuronCores
(512 samples/core). All device compute in bf16 (inputs pre-converted on
host, halving HBM traffic). Activations live feature-major ("x^T":
features on partitions, tokens on the free dim) in an even/odd-pair
dual-stream layout: partitions 0-63 hold even samples' features,
64-127 odd samples'. Dense layers use block-diagonal duplicated weights
so one [128,128]x[128,512] matmul serves both halves at full PE width.
The per-sample adjacency matmuls run as 64x64 quadrant matmuls
(tile_position) with adjacencies pre-transposed on the host; x2 is
flipped to node-major on-chip with PE transpose-mode matmuls.
"""

import sys
import types

import numpy as np
import ml_dtypes

BF16 = ml_dtypes.bfloat16

B, N, S, H = 4096, 64, 64, 64
NCORES = 8
BL = B // NCORES          # 512 samples per core
TS = BL * N // 2          # 16384 token-columns per half
NCHUNK = 32               # chunks per core; 16 samples (1024 tokens) each
GRP = 4                   # chunks per DMA superchunk / action group

_NC_CACHE = {}


def _setup_env():
    """Shim the missing antenv.axon_hooks module so trace=True works."""
    if "antenv.axon_hooks" in sys.modules:
        return
    try:
        from trn_agent_boot.trn_boot import _ntff_profile_via_ctypes
        hook = _ntff_profile_via_ctypes("/opt/axon/libaxon_pjrt.so")
    except Exception:
        hook = None
    mod = types.ModuleType("antenv.axon_hooks")
    mod.get_axon_ntff_profile_hook = lambda: hook
    mod.set_axon_ntff_profile_hook = lambda h: None
    sys.modules["antenv.axon_hooks"] = mod


def _patch_tile():
    """This walrus build rejects instructions with >1 sem wait; split the
    Tile kernel-tail drain's waits across single-wait SP nops."""
    import concourse.mybir as mybir
    import concourse.tile as tile_mod
    from concourse.vector_clock import ScopedClock

    if getattr(tile_mod.TileContext, "_drain_patched", False):
        return

    def _drain_and_barrier(self, tick_clock, wait_clock):
        drain_inst = self.nc.sync.drain()
        wait_clock.add_sem_waits(
            drain_inst.ins, ScopedClock({None: tick_clock.global_clock})
        )
        si = drain_inst.ins.sync_info
        if si is not None and len(si.on_wait) > 1:
            waits = list(si.on_wait)
            drain_inst.ins.sync_info = mybir.SyncInfo(
                on_wait=[waits[0]], on_update=list(si.on_update)
            )
            for w in waits[1:]:
                nop = self.nc.sync.nop()
                nop.ins.sync_info = mybir.SyncInfo(on_wait=[w], on_update=[])
        self.nc.all_engine_barrier()
        popped = self.nc._tile_sem_poison_stack.pop()
        assert popped is self._sem_poison
        self.nc.clear_and_free_semaphores(list(self.sems.allocated().values()))
        self.nc.all_engine_barrier()

    tile_mod.TileContext._drain_and_barrier = _drain_and_barrier
    tile_mod.TileContext._drain_patched = True


def _split_multiwaits(nc):
    """walrus on this stack accepts only one sem-wait per instruction; hoist
    extra waits onto same-engine nops inserted immediately before."""
    import concourse.mybir as mybir

    k = 0
    for bb in nc.m.functions[0].blocks:
        new = []
        for inst in bb.instructions:
            si = inst.sync_info
            if si is not None and len(si.on_wait) > 1:
                waits = list(si.on_wait)
                for w in waits[:-1]:
                    nop = mybir.InstNoOp(
                        name=f"wsplit-{k}",
                        engine=inst.engine,
                        bass_nofuse=True,
                        sync_info=mybir.SyncInfo(on_wait=[w], on_update=[]),
                    )
                    k += 1
                    nc.register_instruction(nop, overwrite=True)
                    new.append(nop)
                inst.sync_info = mybir.SyncInfo(
                    on_wait=[waits[-1]], on_update=list(si.on_update)
                )
            new.append(inst)
        bb.instructions[:] = new


# weight column offsets inside the packed `wts` [128, 1794] bf16 tensor
W1, W2, WC, WO = 0, 128, 256, 384
W3A, W3B, W3C = 512, 640, 768
WIR, WIZ, WIN = 896, 1024, 1152
WHR, WHZ, WHN = 1280, 1408, 1536
IDN, WOUT = 1664, 1792
WCOLS = 1794
# bias column offsets inside `bias` [128, 10] f32
B1, B2, BC, BO, B3, BR, BZ, BHHN, BIHN, BOUT = range(10)


def build_nc(nchunk=NCHUNK, stages="full"):
    import concourse.bass as bass
    import concourse.mybir as mybir
    from concourse.tile import TileContext

    _patch_tile()
    TS_ = 512 * nchunk
    f32 = mybir.dt.float32
    bf16 = mybir.dt.bfloat16
    add = mybir.AluOpType.add
    mult = mybir.AluOpType.mult
    sub = mybir.AluOpType.subtract
    amax = mybir.AluOpType.max
    Sig = mybir.ActivationFunctionType.Sigmoid
    Relu = mybir.ActivationFunctionType.Relu
    Tanh = mybir.ActivationFunctionType.Tanh

    nc = bass.Bass("TRN2")
    xT = nc.declare_dram_parameter("xT", [128, TS_], bf16, isOutput=False)
    hT = nc.declare_dram_parameter("hT", [128, TS_], bf16, isOutput=False)
    adjT = nc.declare_dram_parameter("adjT", [128, 2 * TS_], bf16, isOutput=False)
    wts = nc.declare_dram_parameter("wts", [128, WCOLS], bf16, isOutput=False)
    bias = nc.declare_dram_parameter("bias", [128, 10], f32, isOutput=False)
    houtT = nc.declare_dram_parameter("houtT", [128, TS_], bf16, isOutput=True)
    aout = nc.declare_dram_parameter("aout", [8 * (nchunk // GRP), 512], bf16, isOutput=True)

    with TileContext(nc) as tc:
        with (
            tc.tile_pool(name="const", bufs=1) as cpool,
            tc.tile_pool(name="inp", bufs=2) as ipool,
            tc.tile_pool(name="work", bufs=6) as wpool,
            tc.tile_pool(name="hout", bufs=2) as hpool,
            tc.tile_pool(name="psum", bufs=7, space="PSUM") as ppool,
            tc.tile_pool(name="psumA", bufs=1, space="PSUM") as apool,
        ):
            wsb = cpool.tile([128, WCOLS], bf16)
            nc.sync.dma_start(out=wsb[:], in_=wts[:])
            bsb = cpool.tile([128, 10], f32)
            nc.sync.dma_start(out=bsb[:], in_=bias[:])

            def bias_ap(col):
                return bsb[:, col : col + 1]

            CW = 512 * GRP  # token-cols per superchunk

            for g in range(nchunk // GRP):
                xs = ipool.tile([128, CW], bf16, tag="xs")
                nc.sync.dma_start(out=xs[:], in_=xT[:, g * CW : (g + 1) * CW])
                hs = ipool.tile([128, CW], bf16, tag="hs")
                nc.sync.dma_start(out=hs[:], in_=hT[:, g * CW : (g + 1) * CW])
                ads = ipool.tile([128, 2 * CW], bf16, tag="ads")
                nc.sync.dma_start(
                    out=ads[:], in_=adjT[:, g * 2 * CW : (g + 1) * 2 * CW]
                )
                hacc = hpool.tile([128, CW], bf16, tag="hacc")
                pact = apool.tile([128, 512], f32, tag="pact")
                nc.vector.memset(pact[:], 0.0)

                _psc = [0]

                def ps():
                    _psc[0] += 1
                    return ppool.tile([128, 512], f32, tag="ps", name=f"ps_{g}_{_psc[0]}")

                # ---- Phase 1: fc1/fc2 (weight-stationary x4), transpose, bmm ----
                p1s = [ps() for _ in range(GRP)]
                for k in range(GRP):
                    nc.tensor.matmul(p1s[k][:], wsb[:, W1 : W1 + 128],
                                     xs[:, 512 * k : 512 * (k + 1)], start=True, stop=True)
                x1s = []
                for k in range(GRP):
                    x1 = wpool.tile([128, 512], bf16, tag="x1")
                    nc.scalar.activation(x1[:], p1s[k][:], Relu, bias=bias_ap(B1))
                    x1s.append(x1)
                p2s = [ps() for _ in range(GRP)]
                for k in range(GRP):
                    nc.tensor.matmul(p2s[k][:], wsb[:, W2 : W2 + 128], x1s[k][:], start=True, stop=True)
                x2s = []
                for k in range(GRP):
                    x2 = wpool.tile([128, 512], bf16, tag="x2")
                    nc.scalar.activation(x2[:], p2s[k][:], Relu, bias=bias_ap(B2))
                    x2s.append(x2)
                if stages == "fc2":
                    for k in range(GRP):
                        nc.gpsimd.tensor_copy(out=hacc[:, 512 * k : 512 * (k + 1)], in_=x2s[k][:])
                    nc.sync.dma_start(out=houtT[:, g * CW : (g + 1) * CW], in_=hacc[:])
                    continue

                Msbs = []
                x2nes, x2nos = {}, {}
                for kk in (0, 2):
                    for k in (kk, kk + 1):
                        x2 = x2s[k]
                        pTe = ppool.tile([128, 256], bf16, tag="ps", name=f"pTe_{g}_{k}")
                        pTo = ppool.tile([128, 256], bf16, tag="ps", name=f"pTo_{g}_{k}")
                        for s in range(4):
                            nc.tensor.transpose(
                                pTe[:, 64 * s : 64 * s + 64],
                                x2[0:64, 128 * s : 128 * (s + 1)],
                                wsb[0:64, IDN : IDN + 64],
                                tile_position=(0, 0),
                            )
                            nc.tensor.transpose(
                                pTo[:, 64 * s : 64 * s + 64],
                                x2[64:128, 128 * s : 128 * (s + 1)],
                                wsb[64:128, IDN + 64 : IDN + 128],
                                tile_position=(64, 0),
                            )
                        x2ne = wpool.tile([128, 256], bf16, tag="x2ne")
                        nc.vector.tensor_copy(out=x2ne[:], in_=pTe[:])
                        x2no = wpool.tile([128, 256], bf16, tag="x2no")
                        nc.vector.tensor_copy(out=x2no[:], in_=pTo[:])
                        x2nes[k], x2nos[k] = x2ne, x2no
                    for k in (kk, kk + 1):
                        acol = ads[:, 1024 * k : 1024 * (k + 1)]
                        x2ne, x2no = x2nes[k], x2nos[k]
                        pM0 = ppool.tile([128, 512], f32, tag="ps", name=f"pM0_{g}_{k}")
                        pM1 = ppool.tile([128, 512], f32, tag="ps", name=f"pM1_{g}_{k}")
                        for m in range(16):
                            s, r = m // 4, m % 4
                            rb = 64 * (r // 2)
                            cb = 64 * (r % 2)
                            x2n = x2ne if r % 2 == 0 else x2no
                            pM = pM0 if r < 2 else pM1
                            lhs = x2n[rb : rb + 64, 64 * s : 64 * s + 64]
                            rhs = acol[rb : rb + 64, 256 * s + 128 * (r % 2) : 256 * s + 128 * (r % 2) + 128]
                            nc.tensor.matmul(
                                pM[cb : cb + 64, 128 * s : 128 * s + 128],
                                lhs, rhs, start=True, stop=True,
                                tile_position=(rb, cb),
                            )
                        Msb = wpool.tile([128, 1024], bf16, tag="Msb")
                        M2 = Msb.rearrange("p (a c) -> p a c", a=4, c=256)
                        nc.vector.tensor_copy(out=M2[:, :, 0:128], in_=pM0[:])
                        nc.vector.tensor_copy(out=M2[:, :, 128:256], in_=pM1[:])
                        Msbs.append(Msb)
                if stages in ("xpose", "bmm"):
                    for k in range(GRP):
                        nc.gpsimd.tensor_copy(out=hacc[:, 512 * k : 512 * (k + 1)], in_=Msbs[k][:, 0:512])
                    nc.sync.dma_start(out=houtT[:, g * CW : (g + 1) * CW], in_=hacc[:])
                    continue

                # ---- Phase 2: comp/coop linears + fc3 (weight-stationary x4) ----
                M4s = [Msbs[k].rearrange("p (q two h) -> p q two h", two=2, h=64) for k in range(GRP)]
                pcs = [ps() for _ in range(GRP)]
                for k in range(GRP):
                    nc.tensor.matmul(pcs[k][:], wsb[:, WC : WC + 128], M4s[k][:, :, 0, :], start=True, stop=True)
                xcs = []
                for k in range(GRP):
                    xc = wpool.tile([128, 512], bf16, tag="xc")
                    nc.scalar.activation(xc[:], pcs[k][:], Relu, bias=bias_ap(BC))
                    xcs.append(xc)
                pos = [ps() for _ in range(GRP)]
                for k in range(GRP):
                    nc.tensor.matmul(pos[k][:], wsb[:, WO : WO + 128], M4s[k][:, :, 1, :], start=True, stop=True)
                xos = []
                for k in range(GRP):
                    xo = wpool.tile([128, 512], bf16, tag="xo")
                    nc.vector.tensor_scalar(xo[:], pos[k][:], bias_ap(BO), 0.0, add, amax)
                    xos.append(xo)
                p3s = [ps() for _ in range(GRP)]
                for k in range(GRP):
                    nc.tensor.matmul(p3s[k][:], wsb[:, W3A : W3A + 128], x2s[k][:], start=True, stop=False)
                for k in range(GRP):
                    nc.tensor.matmul(p3s[k][:], wsb[:, W3B : W3B + 128], xcs[k][:], start=False, stop=False)
                for k in range(GRP):
                    nc.tensor.matmul(p3s[k][:], wsb[:, W3C : W3C + 128], xos[k][:], start=False, stop=True)
                x3s = []
                for k in range(GRP):
                    x3 = wpool.tile([128, 512], bf16, tag="x3")
                    nc.vector.tensor_scalar(x3[:], p3s[k][:], bias_ap(B3), 0.0, add, amax)
                    x3s.append(x3)
                if stages == "fc3":
                    for k in range(GRP):
                        nc.gpsimd.tensor_copy(out=hacc[:, 512 * k : 512 * (k + 1)], in_=x3s[k][:])
                    nc.sync.dma_start(out=houtT[:, g * CW : (g + 1) * CW], in_=hacc[:])
                    continue

                # ---- Phase 3: GRU + actions (weight-stationary x4) ----
                hcols = [hs[:, 512 * k : 512 * (k + 1)] for k in range(GRP)]
                prs = [ps() for _ in range(GRP)]
                for k in range(GRP):
                    nc.tensor.matmul(prs[k][:], wsb[:, WIR : WIR + 128], x3s[k][:], start=True, stop=False)
                for k in range(GRP):
                    nc.tensor.matmul(prs[k][:], wsb[:, WHR : WHR + 128], hcols[k], start=False, stop=True)
                rs = []
                for k in range(GRP):
                    r = wpool.tile([128, 512], bf16, tag="r")
                    nc.scalar.activation(r[:], prs[k][:], Sig, bias=bias_ap(BR))
                    rs.append(r)
                pzs = [ps() for _ in range(GRP)]
                for k in range(GRP):
                    nc.tensor.matmul(pzs[k][:], wsb[:, WIZ : WIZ + 128], x3s[k][:], start=True, stop=False)
                for k in range(GRP):
                    nc.tensor.matmul(pzs[k][:], wsb[:, WHZ : WHZ + 128], hcols[k], start=False, stop=True)
                zs = []
                for k in range(GRP):
                    z = wpool.tile([128, 512], bf16, tag="z")
                    nc.scalar.activation(z[:], pzs[k][:], Sig, bias=bias_ap(BZ))
                    zs.append(z)
                # n = tanh(gi_n + bihn + r*(gh_n + bhhn)), two chunks at a time
                ns = []
                for half in range(2):
                    ks = [2 * half, 2 * half + 1]
                    pghs = {k: ps() for k in ks}
                    for k in ks:
                        nc.tensor.matmul(pghs[k][:], wsb[:, WHN : WHN + 128], hcols[k], start=True, stop=True)
                    pgis = {k: ps() for k in ks}
                    for k in ks:
                        nc.tensor.matmul(pgis[k][:], wsb[:, WIN : WIN + 128], x3s[k][:], start=True, stop=True)
                    for k in ks:
                        t2 = wpool.tile([128, 512], bf16, tag="t2")
                        nc.vector.scalar_tensor_tensor(t2[:], pghs[k][:], bias_ap(BHHN), rs[k][:], add, mult)
                        t3 = wpool.tile([128, 512], bf16, tag="t3")
                        nc.vector.tensor_tensor(out=t3[:], in0=t2[:], in1=pgis[k][:], op=add)
                        n = wpool.tile([128, 512], bf16, tag="n")
                        nc.scalar.activation(n[:], t3[:], Tanh, bias=bias_ap(BIHN))
                        ns.append(n)
                for k in range(GRP):
                    u = wpool.tile([128, 512], bf16, tag="u")
                    nc.vector.tensor_tensor(out=u[:], in0=hcols[k], in1=ns[k][:], op=sub)
                    v = wpool.tile([128, 512], bf16, tag="v")
                    nc.gpsimd.tensor_tensor(out=v[:], in0=zs[k][:], in1=u[:], op=mult)
                    nc.gpsimd.tensor_tensor(
                        out=hacc[:, 512 * k : 512 * (k + 1)], in0=ns[k][:], in1=v[:], op=add
                    )
                for k in range(GRP):
                    nc.tensor.matmul(
                        pact[32 * k : 32 * k + 2, :],
                        wsb[:, WOUT : WOUT + 2], x3s[k][:],
                        start=True, stop=True, tile_position=(0, 32 * k),
                    )

                # flush group: h rows and action sigmoid
                nc.sync.dma_start(out=houtT[:, g * CW : (g + 1) * CW], in_=hacc[:])
                asc = wpool.tile([128, 512], bf16, tag="asc")
                nc.scalar.activation(asc[:], pact[:], Sig, bias=bsb[:, BOUT : BOUT + 1])
                for k in range(GRP):
                    nc.sync.dma_start(
                        out=aout[8 * g + 2 * k : 8 * g + 2 * k + 2, :],
                        in_=asc[32 * k : 32 * k + 2, :],
                    )

    _split_multiwaits(nc)
    return nc


def _prep_core_inputs(state, adj_comp, adj_coop, hidden, weights):
    """Build one core's in_map from its f32 shard (all layout work on host)."""
    def eo_T(x):  # [BL,64,64] -> [128, TS] even/odd feature-major
        a = x[0::2].transpose(2, 0, 1).reshape(64, TS)
        b = x[1::2].transpose(2, 0, 1).reshape(64, TS)
        return np.concatenate([a, b], axis=0).astype(BF16)

    acT = adj_comp.transpose(0, 2, 1)
    aoT = adj_coop.transpose(0, 2, 1)
    blk = np.concatenate([acT, aoT], axis=2)  # [BL, 64, 128]
    # sample m = 16c + 4s + 2*rowhalf + q  ->  adjT[64*rh:, 1024c+256s+128q]
    blk = blk.reshape(NCHUNK, 4, 2, 2, 64, 128)
    # adjT.reshape(2, 64, NCHUNK, 4, 2, 128)[rh, j, c, s, q, col]
    adjt = blk.transpose(2, 4, 0, 1, 3, 5).reshape(128, 2 * TS)
    return {
        "xT": eo_T(state),
        "hT": eo_T(hidden),
        "adjT": adjt.astype(BF16),
        "wts": weights["wts"],
        "bias": weights["bias"],
    }


def _prep_weights(fc1_w, fc1_b, fc2_w, fc2_b, comp_w, comp_b, coop_w, coop_b,
                  fc3_w, fc3_b, out_w, out_b, gru_wih, gru_whh, gru_bih, gru_bhh):
    def bd(w):  # blockdiag of transposed layer weight [out,in] -> [128, 128]
        wt = w.T.astype(np.float32)
        z = np.zeros((128, 128), np.float32)
        z[:64, :64] = wt
        z[64:, 64:] = wt
        return z

    cols = np.zeros((128, WCOLS), np.float32)
    cols[:, W1:W1 + 128] = bd(fc1_w)
    cols[:, W2:W2 + 128] = bd(fc2_w)
    cols[:, WC:WC + 128] = bd(comp_w)
    cols[:, WO:WO + 128] = bd(coop_w)
    cols[:, W3A:W3A + 128] = bd(fc3_w[:, 0:64])
    cols[:, W3B:W3B + 128] = bd(fc3_w[:, 64:128])
    cols[:, W3C:W3C + 128] = bd(fc3_w[:, 128:192])
    cols[:, WIR:WIR + 128] = bd(gru_wih[0:64])
    cols[:, WIZ:WIZ + 128] = bd(gru_wih[64:128])
    cols[:, WIN:WIN + 128] = bd(gru_wih[128:192])
    cols[:, WHR:WHR + 128] = bd(gru_whh[0:64])
    cols[:, WHZ:WHZ + 128] = bd(gru_whh[64:128])
    cols[:, WHN:WHN + 128] = bd(gru_whh[128:192])
    cols[:, IDN:IDN + 128] = bd(np.eye(64, dtype=np.float32).T)
    cols[:64, WOUT] = out_w[0]
    cols[64:, WOUT + 1] = out_w[0]

    bias = np.zeros((128, 10), np.float32)
    for col, vec in [(B1, fc1_b), (B2, fc2_b), (BC, comp_b), (BO, coop_b),
                     (B3, fc3_b), (BR, gru_bih[0:64] + gru_bhh[0:64]),
                     (BZ, gru_bih[64:128] + gru_bhh[64:128]),
                     (BHHN, gru_bhh[128:192]), (BIHN, gru_bih[128:192])]:
        bias[:64, col] = vec
        bias[64:, col] = vec
    bias[:, BOUT] = out_b[0]
    return {"wts": cols.astype(BF16), "bias": bias}


def _decode_outputs(results):
    """results: per-core dicts with houtT [128, TS] bf16, aout [64, 512] bf16."""
    h_full = np.empty((B, N, H), np.float32)
    a_full = np.empty((B, N, 1), np.float32)
    for i, res in enumerate(results):
        ht = np.asarray(res["houtT"]).astype(np.float32)
        hs = np.empty((BL, N, H), np.float32)
        hs[0::2] = ht[:64].reshape(64, BL // 2, 64).transpose(1, 2, 0)
        hs[1::2] = ht[64:].reshape(64, BL // 2, 64).transpose(1, 2, 0)
        h_full[i * BL : (i + 1) * BL] = hs
        ao = np.asarray(res["aout"]).astype(np.float32)
        # row = 8g + 2k + parity ; col = 64q + j ; sample = 64g+16k+2q+parity
        ar = ao.reshape(8, 4, 2, 8, 64).transpose(0, 1, 3, 2, 4).reshape(BL, 64)
        a_full[i * BL : (i + 1) * BL, :, 0] = ar
    return a_full, h_full


def kernel(state, adj_comp, adj_coop, hidden,
           fc1_w, fc1_b, fc2_w, fc2_b, comp_w, comp_b, coop_w, coop_b,
           fc3_w, fc3_b, out_w, out_b, gru_wih, gru_whh, gru_bih, gru_bhh,
           trace=False):
    _setup_env()
    from concourse.bass_utils import run_bass_kernel_spmd

    if "nc" not in _NC_CACHE:
        _NC_CACHE["nc"] = build_nc()
    nc = _NC_CACHE["nc"]

    weights = _prep_weights(
        np.asarray(fc1_w), np.asarray(fc1_b), np.asarray(fc2_w), np.asarray(fc2_b),
        np.asarray(comp_w), np.asarray(comp_b), np.asarray(coop_w), np.asarray(coop_b),
        np.asarray(fc3_w), np.asarray(fc3_b), np.asarray(out_w), np.asarray(out_b),
        np.asarray(gru_wih), np.asarray(gru_whh), np.asarray(gru_bih), np.asarray(gru_bhh),
    )
    state = np.asarray(state, np.float32)
    adj_comp = np.asarray(adj_comp, np.float32)
    adj_coop = np.asarray(adj_coop, np.float32)
    hidden = np.asarray(hidden, np.float32)

    in_maps = [
        _prep_core_inputs(
            state[i * BL : (i + 1) * BL], adj_comp[i * BL : (i + 1) * BL],
            adj_coop[i * BL : (i + 1) * BL], hidden[i * BL : (i + 1) * BL], weights
        )
        for i in range(NCORES)
    ]
    res = run_bass_kernel_spmd(nc, in_maps, core_ids=list(range(NCORES)), trace=trace)
    a_full, h_full = _decode_outputs(res.results)
    kernel.last_exec_time_ns = res.exec_time_ns
    return a_full, h_full
